# revision 20
# baseline (speedup 1.0000x reference)
"""Trainium2 Bass kernel for nn_CrossAttentionFusion.

Reference computation (B=16384, img_dim=2048, tab_dim=128, E=256):
    img_p = img_embed @ Wi.T + bi                        (B, E)
    tab_p = tab_embed @ Wt.T + bt                        (B, E)
    img_att = LN(tab_p @ Wc_img.T + bc_img + img_p)      Wc_img = out_w_img @ Wv_img
    tab_att = LN(img_p @ Wc_tab.T + bc_tab + tab_p)
    out     = concat([img_att, tab_att], -1)             (B, 2E)

Host-side algebra (exact) pushes ALL residual adds into PSUM matmul
accumulation so the vector engine only does LN stats + normalize:
  * Wc = out_w @ in_w[2E:]  (softmax over one key is identity).
  * s_img = xi @ Wi.T + xt @ Wfold_img.T        Wfold_img = Wc_img @ Wt
    accumulated in ONE psum tile (16 xi matmuls + 1 xt matmul).
  * s_tab = s_img @ Wc_tab.T + xt @ W_eff.T
    where W_eff.T = Wt.T - (Wc_tab @ Wfold_img).T  (folds the spurious
    xt term introduced by transposing s_img instead of img_p), again one
    psum accumulation (2 transposed-s_img matmuls + 1 xt matmul).
  * out = LN(s_img) | LN(s_tab).

Device plan (pure data parallel, batch sharded 8 ways, weights replicated):
  per 128-row b-tile:
    psA (128,256) = xt.T@WfoldT + sum_k xiT[k].T @ WiT[k]   = s_img
    ACT: cast psA -> s_img_sb (bf16) with accum_out = row-sum
    psT = PE-transpose(s_img_sb);  ACT cast -> s_imgT (bf16)
    psC (128,256) = xt.T@WeffT + sum_et s_imgT[et].T@WcT[et] = s_tab
    DVE: cast psC -> s_tab_sb (bf16) with accum_out = row-sum
    DVE: fused square+reduce for sumsq of both sides
  per slab (4 b-tiles): var/rstd chain (DVE/ACT small ops), then
    normalize (x+negmean)*rstd via DVE tensor_scalar -> bf16 out tile,
    one output DMA per slab.  Output is bf16; host upcasts to f32.

Engine budget per iteration (16 tiles): PE ~36us (bottleneck, 22 matmul
ops/tile), DMA ~31us (xi 8MB + xt 0.5MB in, 2MB bf16 out), ACT ~15us,
DVE ~18us, Pool idle.  Slab 0 runs chunk-major (all four psA groups
interleaved in chunk-arrival order) behind a 24-matmul PE warm-up so the
tensor engine is dense from the first chunk; For_i places an all-engine
barrier between iterations, so per-iteration time is the full single-shot
critical path.  Measured on trn2: 61.0us/iter steady state (loop-delta),
rel err 3.2e-03; previous revision 84.5us (f32r) / 63-66us (bf16).

Scheduling: xi loaded in 512KB chunk tiles on the SP HWDGE ring
(weights/xt/stores on the ACT ring); 3-stage software pipeline
A(j) | T(j-1) | C(j-2); slab widths taper (...,384,128) so the final
output is gated by a small trailing load; ~1.7us of bf16 identity
matmuls open the HAM clock gate during the DMA lead-in.
"""

import json
import os

import numpy as np

E = 256
IMG_DIM = 2048
TAB_DIM = 128
B_FULL = 16384
N_CORES = 8
B_LOC = B_FULL // N_CORES  # 2048
P = 128
KT = IMG_DIM // P  # 16 k-tiles for the img contraction
SLAB = 512
NSLAB = B_LOC // SLAB
TPS = SLAB // P  # 4 b-tiles per slab
EPS = 1e-5

# matmul/data mode: "bf16" (default; rel err ~3e-3), "f32r" (fp32 data,
# fp32r matmuls, rel err ~1.5e-4, ~1.7x slower), "f32" (slowest)
MM_MODE = os.environ.get("KERNEL_MM_MODE", "bf16")

_cache: dict = {}


def _split_multi_waits(bir_bytes: bytes) -> bytes:
    """Work around this walrus build's 1-sync-wait-per-instruction limit.

    Any BIR instruction with >1 `on_wait` fails codegen ("Too many sync wait
    commands").  Hoist all but the last wait onto same-engine EventSemaphore
    instructions inserted immediately before; engines run their stream in
    order, so sequential sem waits are equivalent.
    """
    m = json.loads(bir_bytes)
    for f in m["functions"]:
        for b in f["blocks"]:
            out = []
            for ins in b["instructions"]:
                si = ins.get("sync_info")
                waits = (si or {}).get("on_wait") or []
                if len(waits) > 1:
                    for i, extra in enumerate(waits[:-1]):
                        out.append(
                            {
                                "debug": ins.get("debug", 0),
                                "engine": ins["engine"],
                                "ins": [],
                                "outs": [],
                                "name": f"{ins['name']}-ws{i}",
                                "opcode": "EventSemaphore",
                                "sync_info": {"on_update": [], "on_wait": [extra]},
                            }
                        )
                    si["on_wait"] = [waits[-1]]
                out.append(ins)
            b["instructions"] = out
    return json.dumps(m).encode()


def _build_module(use_bias: bool, use_gb: bool, mode: str, reps: int = 1,
                  unroll: int = 1):
    """reps>1 wraps the body in a hardware loop; unroll>1 python-repeats the
    body with an all-engine barrier between reps (mimics For_i for the
    timeline sim) — both benchmarking only."""
    assert not (reps > 1 and unroll > 1)
    import contextlib

    import concourse.bass as bass
    import concourse.mybir as mybir
    import concourse.tile as tile
    from concourse.masks import make_identity

    f32 = mybir.dt.float32
    xdt = {
        "f32r": mybir.dt.float32r,
        "bf16": mybir.dt.bfloat16,
        "f32": f32,
    }[mode]
    # all PE operands (incl. transposes) stay in one dtype: mixing fp32
    # 2-pass transposes with f32r matmuls intermittently faulted on HW
    tdt = xdt if mode != "f32" else f32
    # output dtype: bf16 in bf16 mode (host upcasts), else f32
    odt = mybir.dt.bfloat16 if mode == "bf16" else f32
    # SBUF dtype of s_img/s_tab working tiles (DVE/ACT read side)
    sdt = xdt if mode != "f32" else f32

    nc = bass.Bass()

    xiT = nc.dram_tensor("xiT", [IMG_DIM, B_LOC], xdt, kind="ExternalInput")
    xtT = nc.dram_tensor("xtT", [TAB_DIM, B_LOC], xdt, kind="ExternalInput")
    wiT = nc.dram_tensor("wiT", [IMG_DIM, E], xdt, kind="ExternalInput")
    # wcat2 cols [0:E) = WfoldT_img, [E:2E) = WeffT
    wcat = nc.dram_tensor("wcat", [TAB_DIM, 2 * E], xdt, kind="ExternalInput")
    wctT = nc.dram_tensor("wctT", [E, E], xdt, kind="ExternalInput")
    out = nc.dram_tensor("out", [B_LOC, 2 * E], odt, kind="ExternalOutput")
    if use_bias:
        # bias rows in matmul dtype, added via rank-1 matmuls
        bias_i = nc.dram_tensor("bias_i", [1, E], xdt, kind="ExternalInput")
        bias_t = nc.dram_tensor("bias_t", [1, E], xdt, kind="ExternalInput")
    if use_gb:
        lng_d = nc.dram_tensor("lng", [E], f32, kind="ExternalInput")
        lnb_d = nc.dram_tensor("lnb", [E], f32, kind="ExternalInput")

    add = mybir.AluOpType.add
    sub = mybir.AluOpType.subtract
    mult = mybir.AluOpType.mult

    def as_f32(ap):
        return ap.bitcast(f32) if ap.dtype == mybir.dt.float32r else ap

    with tile.TileContext(nc) as tc:
        with (
            tc.tile_pool(name="consts", bufs=1) as consts,
            tc.tile_pool(name="xi_pool", bufs=12) as xi_pool,
            tc.tile_pool(name="xt_pool", bufs=3) as xt_pool,
            tc.tile_pool(name="simg", bufs=7) as simg_pool,
            tc.tile_pool(name="stab", bufs=7) as stab_pool,
            tc.tile_pool(name="work", bufs=3) as work,
            tc.tile_pool(name="outp", bufs=3) as outp,
            tc.tile_pool(name="psA", bufs=4, space="PSUM") as psA,
            tc.tile_pool(name="psC", bufs=2, space="PSUM") as psC,
            tc.tile_pool(name="psT", bufs=2, space="PSUM") as psT,
        ):
            # ---- constants ----
            KC = KT // 4
            wiT_r = wiT.rearrange("(t p) e -> p t e", p=P)
            wi_cs = []
            for c in range(4):
                w = consts.tile([P, KC, E], xdt, name=f"wi_c{c}")
                nc.scalar.dma_start(
                    out=w, in_=wiT_r[:, c * KC : (c + 1) * KC, :]
                )
                wi_cs.append(w)
            wcat_sb = consts.tile([P, 2 * E], xdt)
            nc.scalar.dma_start(out=wcat_sb, in_=wcat.ap())
            wct_sb = consts.tile([P, 2, E], xdt)
            nc.scalar.dma_start(out=wct_sb, in_=wctT.rearrange("(t p) e -> p t e", p=P))
            ident_f = consts.tile([P, P], f32)
            make_identity(nc, ident_f)
            if tdt == f32:
                ident = ident_f
            else:
                ident = consts.tile([P, P], tdt)
                nc.scalar.copy(ident, ident_f)
            eps_col = consts.tile([P, 1], f32)
            nc.vector.memset(eps_col, EPS)

            # identity used for in-loop PE warm-up matmuls (see below)
            ident_w = consts.tile([P, P], mybir.dt.bfloat16)
            make_identity(nc, ident_w)
            if use_bias:
                ones_sb = consts.tile([1, P], xdt)
                nc.vector.memset(ones_sb, 1.0)
                bias_i_sb = consts.tile([1, E], xdt)
                nc.scalar.dma_start(out=bias_i_sb, in_=bias_i.ap())
                bias_t_sb = consts.tile([1, E], xdt)
                nc.scalar.dma_start(out=bias_t_sb, in_=bias_t.ap())
            if use_gb:
                lng_sb = consts.tile([P, E], f32)
                nc.sync.dma_start(out=lng_sb, in_=lng_d.ap().to_broadcast((P, E)))
                lnb_sb = consts.tile([P, E], f32)
                nc.sync.dma_start(out=lnb_sb, in_=lnb_d.ap().to_broadcast((P, E)))

            xiT_r = xiT.rearrange("(t p) b -> p t b", p=P)
            out_r = out.rearrange("(t p) e -> p t e", p=P)

            SLAB_W = [512, 512, 512, 384, 128]
            assert sum(SLAB_W) == B_LOC
            slab_b0 = [sum(SLAB_W[:i]) for i in range(len(SLAB_W))]
            btiles = []
            for s, w in enumerate(SLAB_W):
                for j in range(w // P):
                    btiles.append((s, j))
            NB = len(btiles)
            last_jj_of_slab = {s: max(i for i, (s2, _) in enumerate(btiles)
                                      if s2 == s) for s in range(len(SLAB_W))}

            loop_cm = tc.For_i(0, reps, 1) if reps > 1 else contextlib.nullcontext()
            for rep in range(unroll):
              if rep > 0:
                tc.strict_bb_all_engine_barrier()
              with (loop_cm if rep == 0 else contextlib.nullcontext()):
                chunks: dict = {}
                xts: dict = {}
                slab_bufs: dict = {}
                sts: dict = {}

                NCH = [4, 4, 4, 4, 4]

                def load_slab(s):
                    w = SLAB_W[s]
                    kc = KT // NCH[s]
                    bs = slice(slab_b0[s], slab_b0[s] + w)
                    for c in range(NCH[s]):
                        t = xi_pool.tile([P, kc, w], xdt, tag="xi",
                                         name=f"xi{s}_{c}")
                        ks = slice(c * kc, (c + 1) * kc)
                        nc.sync.dma_start(out=t, in_=xiT_r[:, ks, bs])
                        chunks[(s, c)] = t
                    xt = xt_pool.tile([P, w], xdt, tag="xt", name=f"xt{s}")
                    nc.scalar.dma_start(out=xt, in_=xtT[:, bs])
                    xts[s] = xt
                    # per-slab stat rows: [2*j+side] = (img, tab) per tile
                    sum_all = work.tile([P, 2 * TPS], f32, tag="sum_all",
                                        name=f"sum{s}")
                    ssq_all = work.tile([P, 2 * TPS], f32, tag="ssq_all",
                                        name=f"ssq{s}")
                    slab_bufs[s] = (sum_all, ssq_all, {})

                def cast_simg(jj):
                    """ACT cast psA -> SBUF with fused row-sum."""
                    s, j = btiles[jj]
                    sum_all, ssq_all, tiles = slab_bufs[s]
                    pA = sts[jj][0]
                    simg = simg_pool.tile([P, E], sdt, tag="simg",
                                          name=f"simg{jj}")
                    nc.scalar.activation(
                        out=simg, in_=pA,
                        func=mybir.ActivationFunctionType.Copy,
                        accum_out=sum_all[:, 2 * j : 2 * j + 1],
                    )
                    tiles[j] = [simg, None]
                    sts[jj] = (pA, simg)

                def stage1(jj):
                    """psA accumulation (s_img) + ACT cast with row-sum."""
                    s, j = btiles[jj]
                    bcol = slice(j * P, (j + 1) * P)
                    pA = psA.tile([P, E], f32, tag="pA", name=f"pA{jj}")
                    kc = KT // NCH[s]
                    ops = [(xts[s][:, bcol], wcat_sb[:, 0:E])]
                    ops += [
                        (chunks[(s, k // kc)][:, k % kc, bcol],
                         wi_cs[k // KC][:, k % KC, :])
                        for k in range(KT)
                    ]
                    if use_bias:
                        ops.append((ones_sb[:, 0:P], bias_i_sb.ap()))
                    for i, (lhsT, rhs) in enumerate(ops):
                        nc.tensor.matmul(pA, lhsT=lhsT, rhs=rhs,
                                         start=(i == 0), stop=(i == len(ops) - 1))
                    sts[jj] = (pA, None)
                    cast_simg(jj)

                def stage1_slab0_chunk_major():
                    """Slab 0 after the For_i all-engine barrier: PE warm-up
                    fills the first-chunk DMA window, then the four psA
                    accumulation groups interleave chunk-major so PE stays
                    dense while chunks stream in (each chunk is 4 matmuls
                    per b-tile vs ~1.5us of DMA)."""
                    s = 0
                    kc = KT // NCH[s]
                    tps_s = SLAB_W[s] // P
                    warm_ps = psA.tile([P, P], f32, name="warm_ps", tag="pA")
                    for _ in range(24):
                        nc.tensor.matmul(warm_ps, lhsT=ident_w, rhs=ident_w,
                                         start=True, stop=True)
                    pas = []
                    for j in range(tps_s):
                        pA = psA.tile([P, E], f32, tag="pA", name=f"pA{j}")
                        pas.append(pA)
                        nc.tensor.matmul(
                            pA, lhsT=xts[s][:, j * P : (j + 1) * P],
                            rhs=wcat_sb[:, 0:E], start=True, stop=False,
                        )
                    nbias = 1 if use_bias else 0
                    for c in range(NCH[s]):
                        for j in range(tps_s):
                            for k in range(c * kc, (c + 1) * kc):
                                last = (c == NCH[s] - 1 and k == (c + 1) * kc - 1
                                        and not use_bias)
                                nc.tensor.matmul(
                                    pas[j],
                                    lhsT=chunks[(s, c)][:, k % kc,
                                                        j * P : (j + 1) * P],
                                    rhs=wi_cs[k // KC][:, k % KC, :],
                                    start=False, stop=last,
                                )
                            if use_bias:
                                if c == NCH[s] - 1:
                                    nc.tensor.matmul(
                                        pas[j], lhsT=ones_sb[:, 0:P],
                                        rhs=bias_i_sb.ap(),
                                        start=False, stop=True,
                                    )
                            if c == NCH[s] - 1:
                                sts[j] = (pas[j], None)
                                cast_simg(j)

                def stageT(jj):
                    """transpose s_img, cast to matmul dtype, img sumsq."""
                    s, j = btiles[jj]
                    sum_all, ssq_all, tiles = slab_bufs[s]
                    pA, simg = sts[jj]
                    pT = psT.tile([P, E], tdt, tag="pT", name=f"pT{jj}")
                    for et in range(2):
                        nc.tensor.transpose(
                            pT[:, et * P : (et + 1) * P],
                            simg[:, et * P : (et + 1) * P],
                            ident,
                        )
                    simgT = work.tile([P, E], xdt, tag="simgT", name=f"simgT{jj}")
                    nc.scalar.copy(simgT, pT)
                    simg_f = as_f32(simg)
                    scr = work.tile([P, E], sdt, tag="scr0")
                    nc.vector.scalar_tensor_tensor(
                        out=scr, in0=simg_f, scalar=1.0, in1=simg_f,
                        op0=mult, op1=mult,
                        accum_out=ssq_all[:, 2 * j : 2 * j + 1],
                    )
                    sts[jj] = (pA, simg, simgT)

                def stageC(jj):
                    """psC accumulation (s_tab) + DVE cast/sum + tab sumsq."""
                    s, j = btiles[jj]
                    sum_all, ssq_all, tiles = slab_bufs[s]
                    pA, simg, simgT = sts[jj]
                    pC = psC.tile([P, E], f32, tag="pC", name=f"pC{jj}")
                    ops = [(xts[s][:, j * P : (j + 1) * P], wcat_sb[:, E : 2 * E])]
                    ops += [
                        (simgT[:, et * P : (et + 1) * P], wct_sb[:, et, :])
                        for et in range(2)
                    ]
                    if use_bias:
                        ops.append((ones_sb[:, 0:P], bias_t_sb.ap()))
                    for i, (lhsT, rhs) in enumerate(ops):
                        nc.tensor.matmul(pC, lhsT=lhsT, rhs=rhs,
                                         start=(i == 0), stop=(i == len(ops) - 1))
                    stab = stab_pool.tile([P, E], sdt, tag="stab",
                                          name=f"stab{jj}")
                    # with accum_out, op1 is the REDUCE op (TensorScalarPtrReduce):
                    # out = in0 + 0.0 (cast), accum_out = sum(out) + 0.0
                    nc.vector.tensor_scalar(
                        out=stab, in0=pC, scalar1=0.0, scalar2=0.0, op0=add,
                        op1=add,
                        accum_out=sum_all[:, 2 * j + 1 : 2 * j + 2],
                    )
                    stab_f = as_f32(stab)
                    scr = work.tile([P, E], sdt, tag="scr1")
                    nc.vector.scalar_tensor_tensor(
                        out=scr, in0=stab_f, scalar=1.0, in1=stab_f,
                        op0=mult, op1=mult,
                        accum_out=ssq_all[:, 2 * j + 1 : 2 * j + 2],
                    )
                    tiles[j][1] = stab

                def epilogue(s):
                    """LN stats chain + normalize + output DMA for slab s."""
                    tps_s = SLAB_W[s] // P
                    n = 2 * tps_s
                    sum_all, ssq_all, tiles = slab_bufs[s]
                    m_neg = work.tile([P, 2 * TPS], f32, tag="m_neg")
                    nc.vector.tensor_scalar(
                        out=m_neg[:, 0:n], in0=sum_all[:, 0:n],
                        scalar1=-1.0 / E, scalar2=None, op0=mult,
                    )
                    var = work.tile([P, 2 * TPS], f32, tag="var")
                    # var = ssq/E - mean^2  (mean^2 == m_neg^2)
                    nc.vector.tensor_scalar(
                        out=var[:, 0:n], in0=ssq_all[:, 0:n],
                        scalar1=1.0 / E, scalar2=None, op0=mult,
                    )
                    msq = work.tile([P, 2 * TPS], f32, tag="msq")
                    nc.vector.tensor_tensor(
                        out=msq[:, 0:n], in0=m_neg[:, 0:n], in1=m_neg[:, 0:n],
                        op=mult,
                    )
                    nc.vector.tensor_tensor(
                        out=var[:, 0:n], in0=var[:, 0:n], in1=msq[:, 0:n],
                        op=sub,
                    )
                    sd = work.tile([P, 2 * TPS], f32, tag="sd")
                    nc.scalar.activation(
                        out=sd[:, 0:n], in_=var[:, 0:n],
                        func=mybir.ActivationFunctionType.Sqrt,
                        bias=eps_col, scale=1.0,
                    )
                    rstd = work.tile([P, 2 * TPS], f32, tag="rstd")
                    nc.vector.reciprocal(rstd[:, 0:n], sd[:, 0:n])
                    t0 = slab_b0[s] // P
                    o_slab = outp.tile([P, TPS, 2 * E], odt, tag="o")
                    for j in range(tps_s):
                        simg, stab = tiles[j]
                        for side, src in ((0, as_f32(simg)), (1, as_f32(stab))):
                            o_slice = o_slab[:, j, side * E : (side + 1) * E]
                            dst = o_slice
                            if use_gb:
                                dst = work.tile([P, E], f32, tag=f"n{side}")
                            idx = 2 * j + side
                            nc.vector.tensor_scalar(
                                out=dst, in0=src,
                                scalar1=m_neg[:, idx : idx + 1],
                                scalar2=rstd[:, idx : idx + 1],
                                op0=add, op1=mult,
                            )
                            if use_gb:
                                scaled = work.tile([P, E], f32, tag=f"sc{side}")
                                nc.gpsimd.tensor_mul(scaled, dst, lng_sb)
                                nc.gpsimd.tensor_add(o_slice, scaled, lnb_sb)
                    nc.scalar.dma_start(
                        out=out_r[:, t0 : t0 + tps_s, :],
                        in_=o_slab[:, 0:tps_s, :],
                    )

                TAIL_JJ = last_jj_of_slab[len(SLAB_W) - 3] + 1

                def maybe_epilogue(x):
                    s2 = btiles[x][0]
                    if x == last_jj_of_slab[s2]:
                        epilogue(s2)

                loaded = {0}
                load_slab(0)
                stage1_slab0_chunk_major()
                # transition: catch the T/C pipeline up to the generic
                # stage1(jj) | T(jj-1) | C(jj-2) pattern entered at jj=4
                stageT(0)
                stageT(1)
                stageC(0)
                stageT(2)
                stageC(1)
                for jj in range(TPS, NB):
                    s_cur = btiles[jj][0]
                    if s_cur not in loaded:
                        loaded.add(s_cur)
                        load_slab(s_cur)
                    stage1(jj)
                    if jj < TAIL_JJ:
                        stageT(jj - 1)
                        stageC(jj - 2)
                        maybe_epilogue(jj - 2)
                    else:
                        if jj == TAIL_JJ:
                            stageT(jj - 1)
                            stageC(jj - 2)
                            maybe_epilogue(jj - 2)
                            stageC(jj - 1)
                            maybe_epilogue(jj - 1)
                        stageT(jj)
                        stageC(jj)
                        maybe_epilogue(jj)

    return nc


def _prep_inputs(inputs: dict, mode: str):
    """Host-side shard + weight folding. Returns (in_maps, use_bias, use_gb)."""
    import ml_dtypes

    f = lambda k: np.asarray(inputs[k], dtype=np.float64)
    Wi, bi = f("Wi"), f("bi")
    Wt, bt = f("Wt"), f("bt")
    Wc_img = f("out_w_img") @ f("in_w_img")[2 * E :]
    bc_img = f("out_w_img") @ f("in_b_img")[2 * E :] + f("out_b_img")
    Wc_tab = f("out_w_tab") @ f("in_w_tab")[2 * E :]
    bc_tab = f("out_w_tab") @ f("in_b_tab")[2 * E :] + f("out_b_tab")

    Wfold_img = Wc_img @ Wt                      # (E, TAB_DIM)
    Weff_T = Wt.T - (Wc_tab @ Wfold_img).T       # (TAB_DIM, E)
    bias_img = bi + Wc_img @ bt + bc_img
    bias_tab = bt + bc_tab + Wc_tab @ (bi - bias_img)

    lng = np.asarray(inputs["ln_g"], dtype=np.float32)
    lnb = np.asarray(inputs["ln_b"], dtype=np.float32)
    use_bias = bool(np.any(bias_img != 0.0) or np.any(bias_tab != 0.0))
    use_gb = bool(np.any(lng != 1.0) or np.any(lnb != 0.0))

    xdt = ml_dtypes.bfloat16 if mode == "bf16" else np.float32
    wiT = np.ascontiguousarray(Wi.T).astype(xdt)
    wcat = np.concatenate([Wfold_img.T, Weff_T], axis=1).astype(xdt)  # (128,512)
    wctT = np.ascontiguousarray(Wc_tab.T).astype(xdt)

    xi = np.asarray(inputs["img_embed"], dtype=np.float32)
    xt = np.asarray(inputs["tab_embed"], dtype=np.float32)
    xiT = np.ascontiguousarray(xi.T).astype(xdt)  # (IMG_DIM, B)
    xtT = np.ascontiguousarray(xt.T).astype(xdt)  # (TAB_DIM, B)

    in_maps = []
    for c in range(N_CORES):
        bs = slice(c * B_LOC, (c + 1) * B_LOC)
        m = {
            "xiT": np.ascontiguousarray(xiT[:, bs]),
            "xtT": np.ascontiguousarray(xtT[:, bs]),
            "wiT": wiT,
            "wcat": wcat,
            "wctT": wctT,
        }
        if use_bias:
            m["bias_i"] = bias_img.reshape(1, E).astype(xdt)
            m["bias_t"] = bias_tab.reshape(1, E).astype(xdt)
        if use_gb:
            m["lng"] = lng
            m["lnb"] = lnb
        in_maps.append(m)
    return in_maps, use_bias, use_gb


def _kernel_impl(inputs: dict, trace: bool):
    from concourse.bass_utils import run_bass_kernel_spmd

    mode = MM_MODE
    in_maps, use_bias, use_gb = _prep_inputs(inputs, mode)
    key = (use_bias, use_gb, mode)
    if key not in _cache:
        nc = _build_module(use_bias, use_gb, mode)
        orig = nc.to_json_bytes
        nc.to_json_bytes = lambda: _split_multi_waits(orig())
        _cache[key] = nc
    nc = _cache[key]

    try:
        res = run_bass_kernel_spmd(
            nc,
            in_maps,
            core_ids=list(range(N_CORES)),
            trace=trace,
            trace_cores=[0] if trace else None,
        )
    except ModuleNotFoundError:
        res = run_bass_kernel_spmd(nc, in_maps, core_ids=list(range(N_CORES)))
    out = np.concatenate(
        [np.asarray(r["out"]).astype(np.float32) for r in res.results], axis=0
    )
    return out, res


def kernel(**inputs) -> np.ndarray:
    out, _ = _kernel_impl(inputs, trace=False)
    return out


def kernel_traced(**inputs):
    return _kernel_impl(inputs, trace=True)
